# revision 5
# baseline (speedup 1.0000x reference)
"""LocalAttention (B=1, S=4096, D=1024, H=16, hd=64, window=128) on 8 trn2 cores.

Sharding: sequence-parallel. Core c owns queries [512c, 512c+512) and receives
a key/value halo slice of 768 rows ([512c-128, 512c+640), zero-padded at the
global edges). All projection weights are replicated (bf16). Everything on
device runs in bf16 with fp32 PSUM accumulation.

v2 per-core dataflow (changes vs v1 in *):
  * PE warm-up: ~10 dummy matmuls on a zeroed tile right at kernel start so
    the HAM clock-gate is at 8/8 when the first real projection fires.
  * Batched input DMAs (one or two per tensor, 3-D APs) issued on the two
    HWDGE queues (Sync/Scalar) -- ~12 DIRECT2Ds instead of ~73.
  qT = (Wq^T x^T) in [e, s] layout; kT likewise over the 768-col halo range;
  v = (Vin Wv) in natural [s, e] layout + a ones-column per head.
  * Scores issued per HEAD-PAIR: the two heads of an e-block live at
    partitions 0-63 / 64-127, so their K=64 matmuls go to different PE
    row-groups and run concurrently (auto tile_position via base partition).
  Exp (scale=1/8, bf16, no max-subtract) on ACT; *triangle masks on DVE.
  * PV is v-STATIONARY: out[e0:65, q] = sum_kb (v_kb|1).T @ expp_kb -- the
    attention output lands directly in [e, q] (transposed) layout with the
    softmax denominator on partition 64.  No XBAR DMA transposes at all.
  * Normalize: DVE reciprocal of the denom row -> GpSimd partition_broadcast
    [64,128] -> DVE multiply into the aot tile half (odd heads write
    partitions 64-127; 64-channel DVE ops may retarget quadrants 2/3).
  * Output projection interleaved per-head into the qb loop (chain MM for
    e-block eb issues two heads after aot[eb] is complete), so the last-qb
    tail is ~2 matmuls + split copies + DMA instead of transpose+chain.
  Out staging copies split [128,256] across Vector and Scalar engines.
"""

import os

import numpy as np
import ml_dtypes

import concourse.bass as bass
import concourse.bacc as bacc
import concourse.mybir as mybir
import concourse.tile as tile
from concourse.bass_utils import run_bass_kernel_spmd

BF16 = mybir.dt.bfloat16
FP32 = mybir.dt.float32

NCORES = 8
S = 4096
D = 1024
H = 16
HD = 64
E = H * HD  # 1024
WIN = 128
SL = S // NCORES       # 512 queries per core
SK = SL + 2 * WIN      # 768 keys/values incl. halo
NQB = SL // 128        # 4 query blocks
NKB = SK // 128        # 6 key blocks
NDB = D // 128         # 8 contraction blocks
NEB = E // 128         # 8 embed blocks
VROW = HD + 1          # 65: v columns per head incl. ones column

# kb-major score tiles: valid q-blocks for key-block kb are
# [max(0, kb-2), min(NQB-1, kb)] (window = +-1 block around diagonal).
KB_Q0 = [max(0, kb - 2) for kb in range(NKB)]
KB_QN = [min(NQB - 1, kb) - max(0, kb - 2) + 1 for kb in range(NKB)]

_CACHE = {}
LAST_RESULT = None  # BassKernelResults of the most recent run (for test.py)


def _build_nc():
    nc = bacc.Bacc("TRN2", target_bir_lowering=False, debug=False)

    qt_d = nc.dram_tensor("qt", [D, SL], BF16, kind="ExternalInput").ap()
    kt_d = nc.dram_tensor("kt", [D, SK], BF16, kind="ExternalInput").ap()
    vt_d = nc.dram_tensor("vt", [D, SK], BF16, kind="ExternalInput").ap()
    wq_d = nc.dram_tensor("wq", [D, E], BF16, kind="ExternalInput").ap()
    wk_d = nc.dram_tensor("wk", [D, E], BF16, kind="ExternalInput").ap()
    wv_d = nc.dram_tensor("wv", [D, E], BF16, kind="ExternalInput").ap()
    wo_d = nc.dram_tensor("wo", [E, D], BF16, kind="ExternalInput").ap()
    # bf16 multiplicative mask pairs (one strided DVE op covers blocks {0,2}
    # of a [128,384] score tile): [0:256]=[mk0|m0] for the kb0+kb1 pair,
    # [256:512]=[m2|m0] for kb2/kb3, [512:768]=[m2|mk5] for the kb4+kb5
    # pair. mk0/mk5 are m0/m2 with the global-edge zero blocks baked in.
    msk_d = nc.dram_tensor("msk", [128, 768], BF16, kind="ExternalInput").ap()
    out_d = nc.dram_tensor("out", [SL, D], FP32, kind="ExternalOutput").ap()

    with tile.TileContext(nc) as tc:
        pools = []

        def pool(name, bufs, **kw):
            p = tc.tile_pool(name=name, bufs=bufs, **kw)
            pools.append(p)
            return p.__enter__()

        const = pool("const", 1)
        psum = pool("psum", 2, space="PSUM")       # projections + out proj (+vch tag)
        pscore = pool("pscore", 3, space="PSUM")   # score tiles [128, 384]
        ppv_pool = pool("ppv", 2, space="PSUM")    # PV tiles [65, 128]
        ep = pool("expp", 3)                       # per-head exp tiles
        aotp = pool("aot", 16)                     # per-(qb,eb) [e, q] bf16
        op = pool("o", 4)                          # fp32 out staging [128,256]
        rp = pool("recip", 4)                      # [1,128] recip rows
        rbp = pool("rbc", 4)                       # [64,128] broadcast recips

        # ---- persistent SBUF tensors ----
        warm_sb = const.tile([128, 512], BF16, tag="warm")
        wq_sb = const.tile([128, NDB * E], BF16, tag="wq")
        wk_sb = const.tile([128, NDB * E], BF16, tag="wk")
        wv_sb = const.tile([128, NDB * E], BF16, tag="wv")
        wo_sb = const.tile([128, NEB * D], BF16, tag="wo")
        qtin_sb = const.tile([128, NDB * SL], BF16, tag="qtin")
        ktin_sb = const.tile([128, NDB * SK], BF16, tag="ktin")
        vtin_sb = const.tile([128, NDB * SK], BF16, tag="vtin")
        qt_sb = const.tile([128, NEB * SL], BF16, tag="qt")    # [e,s] per e-blk
        kt_sb = const.tile([128, NEB * SK], BF16, tag="kt")
        v_sb = const.tile([128, NKB * H * VROW], BF16, tag="v")  # [s, h*65]
        msk_sb = const.tile([128, 768], BF16, tag="msk")

        sync = nc.sync

        # ---- PE warm-up: ~4us of dummy matmuls so the HAM clock-gate is
        # already 8/8 when the first data-dependent matmul issues ----
        nc.gpsimd.memset(warm_sb[:], 0.0)
        wps = psum.tile([128, 512], FP32, tag="ps")
        for _ in range(10):
            nc.tensor.matmul(
                wps[:], lhsT=warm_sb[:, 0:128], rhs=warm_sb[:],
                start=True, stop=True,
            )

        # ones columns of v_sb (col hd=64 of each head group)
        v3 = v_sb[:].rearrange("p (k h c) -> p k h c", k=NKB, h=H)
        nc.gpsimd.memset(v3[:, :, :, HD:VROW], 1.0)

        # ---- input DMAs: batched 3-D APs on the two HWDGE queues, ordered
        # as the projections consume them ----
        def slabs(dr, ncols):
            return dr.rearrange("(b p) c -> p b c", p=128)

        def sb3(sb, ncols):
            return sb[:].rearrange("p (b c) -> p b c", c=ncols)

        # first Q-proj chain needs qtin slab0 + wq[:, eb0] slabs
        sync.dma_start(qtin_sb[:, 0:SL], qt_d[0:128, :])
        nc.scalar.dma_start(wq_sb[:, 0:128], wq_d[0:128, 0:128])
        nc.scalar.dma_start(
            sb3(wq_sb, E)[:, 1:NDB, 0:128], slabs(wq_d, E)[:, 1:NDB, 0:128]
        )
        sync.dma_start(sb3(qtin_sb, SL)[:, 1:NDB], slabs(qt_d, SL)[:, 1:NDB])
        nc.scalar.dma_start(
            sb3(wq_sb, E)[:, :, 128:E], slabs(wq_d, E)[:, :, 128:E]
        )
        sync.dma_start(sb3(ktin_sb, SK)[:], slabs(kt_d, SK)[:])
        nc.scalar.dma_start(sb3(wk_sb, E)[:], slabs(wk_d, E)[:])
        sync.dma_start(msk_sb[:], msk_d[:])
        nc.scalar.dma_start(sb3(vtin_sb, SK)[:], slabs(vt_d, SK)[:])
        sync.dma_start(sb3(wv_sb, E)[:], slabs(wv_d, E)[:])
        nc.scalar.dma_start(sb3(wo_sb, D)[:], slabs(wo_d, D)[:])

        # ---- q projection: [e, s] = Wq[d,e].T @ QT[d,s] ----
        for eb in range(NEB):
            ps = psum.tile([128, 512], FP32, tag="ps")
            for db in range(NDB):
                nc.tensor.matmul(
                    ps[:],
                    lhsT=wq_sb[:, db * E + eb * 128: db * E + (eb + 1) * 128],
                    rhs=qtin_sb[:, db * SL: db * SL + SL],
                    start=(db == 0),
                    stop=(db == NDB - 1),
                )
            nc.vector.tensor_copy(qt_sb[:, eb * SL:(eb + 1) * SL], ps[:])

        # ---- k projection: [e, s] = Wk[d,e].T @ KT[d,s] over halo range ----
        for eb in range(NEB):
            for s0, s1 in ((0, 512), (512, SK)):
                ps = psum.tile([128, 512], FP32, tag="ps")
                for db in range(NDB):
                    nc.tensor.matmul(
                        ps[:, : s1 - s0],
                        lhsT=wk_sb[:, db * E + eb * 128: db * E + (eb + 1) * 128],
                        rhs=ktin_sb[:, db * SK + s0: db * SK + s1],
                        start=(db == 0),
                        stop=(db == NDB - 1),
                    )
                nc.vector.tensor_copy(
                    kt_sb[:, eb * SK + s0: eb * SK + s1], ps[:, : s1 - s0]
                )

        # ---- v projection chain: one (kb, eh) psum chain ----
        def v_chain(kb, eh):
            ps = psum.tile([128, 512], FP32, tag="vch", bufs=1)
            for db in range(NDB):
                nc.tensor.matmul(
                    ps[:],
                    lhsT=vtin_sb[:, db * SK + kb * 128: db * SK + (kb + 1) * 128],
                    rhs=wv_sb[:, db * E + eh * 512: db * E + (eh + 1) * 512],
                    start=(db == 0),
                    stop=(db == NDB - 1),
                )
            dst = v3[:, kb, eh * 8:(eh + 1) * 8, 0:HD]
            src = ps[:].rearrange("p (h c) -> p h c", c=HD)
            nc.scalar.copy(dst, src)

        # ---- attention pieces ----
        scale = 1.0 / np.sqrt(HD)
        expp_tiles = {}  # (h, kb) -> (sbuf tile, col base)

        def score_pair(j, kbs, mskoff):
            """Score matmuls for head pair (2j, 2j+1) over key blocks kbs.
            The two heads sit at partitions 0-63 / 64-127 of e-block j, so
            per kb their two K=64 matmuls run in different PE row groups
            (concurrently). One exp + one strided mask multiply per head."""
            pscrs = (
                pscore.tile([128, 384], FP32, tag="scr", name=f"scr_{2*j}_{kbs[0]}"),
                pscore.tile([128, 384], FP32, tag="scr", name=f"scr_{2*j+1}_{kbs[0]}"),
            )
            col = 0
            for kb in kbs:
                w = KB_QN[kb] * 128
                q0 = KB_Q0[kb] * 128
                for t in range(2):
                    hp = t * HD
                    nc.tensor.matmul(
                        pscrs[t][:, col:col + w],
                        lhsT=kt_sb[hp:hp + HD,
                                   j * SK + kb * 128: j * SK + (kb + 1) * 128],
                        rhs=qt_sb[hp:hp + HD, j * SL + q0: j * SL + q0 + w],
                        start=True,
                        stop=True,
                    )
                col += w
            for t in range(2):
                h = 2 * j + t
                expp = ep.tile([128, 384], BF16, tag=f"expp{h}",
                               name=f"expp_{h}_{kbs[0]}")
                nc.scalar.activation(
                    expp[:], pscrs[t][:],
                    mybir.ActivationFunctionType.Exp, scale=scale,
                )
                # masked 128-col blocks are always local blocks {0, 2}
                ev = expp[:].rearrange("p (b c) -> p b c", b=3)
                dst = ev[:, 0::2]
                src = msk_sb[:, mskoff:mskoff + 256].rearrange(
                    "p (b c) -> p b c", b=2)
                nc.vector.tensor_mul(dst, dst, src)
                col = 0
                for kb in kbs:
                    expp_tiles[(h, kb)] = (expp, col)
                    col += KB_QN[kb] * 128

        def pv_norm(h, qb, aot):
            """v-stationary PV for one (head, q-block): out [65, 128] in
            [e, q] layout with the softmax denominator on partition 64;
            then reciprocal -> partition broadcast -> multiply into the
            (h%2) half of aot[h//2]."""
            ppv = ppv_pool.tile([VROW, 128], FP32, tag="pv")
            for r in range(3):
                kb = qb + r
                tile_, base = expp_tiles[(h, kb)]
                off = base + (qb - KB_Q0[kb]) * 128
                nc.tensor.matmul(
                    ppv[:],
                    lhsT=v_sb[:, (kb * H + h) * VROW:(kb * H + h + 1) * VROW],
                    rhs=tile_[:, off:off + 128],
                    start=(r == 0),
                    stop=(r == 2),
                )
            rsb = rp.tile([1, 128], FP32, tag="rd")
            nc.vector.reciprocal(rsb[:], ppv[HD:VROW, :])
            rbc = rbp.tile([HD, 128], FP32, tag="rb")
            nc.gpsimd.partition_broadcast(rbc[:], rsb[:])
            dst = aot[h // 2][(h % 2) * HD:(h % 2) * HD + HD, :]
            nc.vector.tensor_mul(dst, ppv[0:HD, :], rbc[:])

        # ---- prologue attention: kb0+kb1 pair tiles + V kb0..2 ----
        for j in range(H // 2):
            score_pair(j, (0, 1), 0)
            if j == 1:
                v_chain(0, 0)
            if j == 3:
                v_chain(0, 1)
            if j == 5:
                v_chain(1, 0)
            if j == 7:
                v_chain(1, 1)
        v_chain(2, 0)
        v_chain(2, 1)

        # score groups computed per qb iteration (kb4+kb5 paired at qb2)
        KBN_GROUPS = {0: ((2,), 256), 1: ((3,), 256), 2: ((4, 5), 512)}

        # ---- rolling qb loop with inline per-head output projection ----
        for qb in range(NQB):
            aot = [
                aotp.tile([128, 128], BF16, tag="aot", name=f"aot_{qb}_{eb}")
                for eb in range(NEB)
            ]
            grp = KBN_GROUPS.get(qb)
            pso = [None, None]

            def opj_mm(eb):
                for dh in range(2):
                    if pso[dh] is None:
                        pso[dh] = psum.tile([128, 512], FP32, tag="ps",
                                            name=f"opj_{qb}_{dh}")
                    nc.tensor.matmul(
                        pso[dh][:],
                        lhsT=aot[eb][:],
                        rhs=wo_sb[:, eb * D + dh * 512: eb * D + (dh + 1) * 512],
                        start=(eb == 0),
                        stop=(eb == NEB - 1),
                    )

            for h in range(H):
                if grp is not None and h % 2 == 0:
                    score_pair(h // 2, grp[0], grp[1])
                if qb < NQB - 1 and h in (4, 10):
                    v_chain(qb + 3, 0 if h == 4 else 1)
                if h >= 1:
                    pv_norm(h - 1, qb, aot)
                # aot[eb] is complete two heads back -> safe to chain now
                if h % 2 == 1 and h >= 3:
                    opj_mm((h - 3) // 2)
            pv_norm(H - 1, qb, aot)
            opj_mm(NEB - 1)
            for dh in range(2):
                for half in range(2):
                    o_t = op.tile([128, 256], FP32, tag="o")
                    src = pso[dh][:, half * 256:(half + 1) * 256]
                    if half == 0:
                        nc.vector.tensor_copy(o_t[:], src)
                    else:
                        nc.scalar.copy(o_t[:], src)
                    eng = sync if half == 0 else nc.scalar
                    eng.dma_start(
                        out_d[qb * 128:(qb + 1) * 128,
                              dh * 512 + half * 256: dh * 512 + (half + 1) * 256],
                        o_t[:],
                    )

        for p in reversed(pools):
            p.__exit__(None, None, None)

    nc.compile()
    return nc


def _host_masks():
    bf = ml_dtypes.bfloat16
    kt = np.arange(128)[:, None]
    qi = np.arange(128)[None, :]
    tri0 = (qi <= kt).astype(bf)          # r=0 keep
    tri2 = (kt <= qi).astype(bf)          # r=2 keep
    zeros = np.zeros((128, 128), bf)

    masks = []
    for c in range(NCORES):
        m = np.empty((128, 768), bf)
        m[:, 0:128] = zeros if c == 0 else tri0           # mk0
        m[:, 128:256] = tri0                              # m0 (kb1 pair)
        m[:, 256:384] = tri2                              # m2 (kb2/kb3)
        m[:, 384:512] = tri0                              # m0 (kb2/kb3)
        m[:, 512:640] = tri2                              # m2 (kb4 pair)
        m[:, 640:768] = zeros if c == NCORES - 1 else tri2  # mk5
        masks.append(m)
    return masks


def _host_inputs(query, key, value, Wq, Wk, Wv, Wo):
    bf = ml_dtypes.bfloat16
    q2 = np.ascontiguousarray(query.reshape(S, D))
    k2 = np.asarray(key).reshape(S, D)
    v2 = np.asarray(value).reshape(S, D)
    kpad = np.zeros((S + 2 * WIN, D), np.float32)
    kpad[WIN:WIN + S] = k2
    vpad = np.zeros((S + 2 * WIN, D), np.float32)
    vpad[WIN:WIN + S] = v2

    wq = np.ascontiguousarray(Wq.astype(bf))
    wk = np.ascontiguousarray(Wk.astype(bf))
    wv = np.ascontiguousarray(Wv.astype(bf))
    wo = np.ascontiguousarray(Wo.astype(bf))
    masks = _host_masks()

    in_maps = []
    for c in range(NCORES):
        s0 = c * SL
        qt = np.ascontiguousarray(q2[s0:s0 + SL].T.astype(bf))
        ktc = np.ascontiguousarray(kpad[s0:s0 + SK].T.astype(bf))
        vtc = np.ascontiguousarray(vpad[s0:s0 + SK].T.astype(bf))
        in_maps.append({
            "qt": qt, "kt": ktc, "vt": vtc,
            "wq": wq, "wk": wk, "wv": wv, "wo": wo,
            "msk": masks[c],
        })
    return in_maps


def kernel(query, key, value, Wq, Wk, Wv, Wo):
    global LAST_RESULT
    if "nc" not in _CACHE:
        _CACHE["nc"] = _build_nc()
    nc = _CACHE["nc"]
    in_maps = _host_inputs(
        np.asarray(query), np.asarray(key), np.asarray(value),
        np.asarray(Wq), np.asarray(Wk), np.asarray(Wv), np.asarray(Wo),
    )
    trace = os.environ.get("KERNEL_TRACE", "0") == "1"
    try:
        res = run_bass_kernel_spmd(
            nc, in_maps, core_ids=list(range(NCORES)), trace=trace
        )
    except ModuleNotFoundError:
        res = run_bass_kernel_spmd(
            nc, in_maps, core_ids=list(range(NCORES)), trace=False
        )
    LAST_RESULT = res
    out = np.concatenate([res.results[c]["out"] for c in range(NCORES)], axis=0)
    return out.reshape(1, S, D).astype(np.float32)


# revision 6
# speedup vs baseline: 1.2030x; 1.2030x over previous
"""LocalAttention (B=1, S=4096, D=1024, H=16, hd=64, window=128) on 8 trn2 cores.

Sharding: sequence-parallel. Core c owns queries [512c, 512c+512) and receives
a key/value halo slice of 768 rows ([512c-128, 512c+640), zero-padded at the
global edges). All projection weights are replicated (bf16). Everything on
device runs in bf16 with fp32 PSUM accumulation.

v3 per-core dataflow:
  PE warm-up (~10 dummy matmuls on a zeroed tile) so the HAM clock gate is
  8/8 when the first real projection fires.  Batched input DMAs (one or two
  3-D-AP DMAs per tensor) on the two HWDGE queues.
  qT = (Wq^T x^T) in [e, s] layout; kT likewise over the 768-col halo range;
  v = (Vin Wv) in natural [s, e] layout + a ones-column per head (softmax
  denominator rides along col 64 of each head's 65-wide v group).
  Scores issued per HEAD-PAIR: the two heads of an e-block live at
  partitions 0-63 / 64-127, so their K=64 matmuls target different PE
  row-groups and run concurrently.  Exp (scale=1/8, bf16, no max-subtract)
  on ACT; triangle masks on GpSimd.
  PV per (head, q-block): 3 accumulating matmuls lhsT=expp slice,
  rhs=[v_h | 1] -> [128, 65]; DVE reciprocal + tensor_scalar normalize into
  ao ([q, e] bf16).  ao -> aot via PER-EBLOCK [128,128] XBAR transposes
  issued as soon as the e-block's two heads are normalized, and the output
  projection is interleaved per-head into the same loop (chain matmul for
  e-block eb issues three heads after its transpose), so the last-qb tail is
  one transpose + two chain matmuls + split copies + DMA.
  Out staging copies are split [128,256] across Vector and Scalar engines.
"""

import os

import numpy as np
import ml_dtypes

import concourse.bass as bass
import concourse.bacc as bacc
import concourse.mybir as mybir
import concourse.tile as tile
from concourse.bass_utils import run_bass_kernel_spmd

BF16 = mybir.dt.bfloat16
FP32 = mybir.dt.float32

NCORES = 8
S = 4096
D = 1024
H = 16
HD = 64
E = H * HD  # 1024
WIN = 128
SL = S // NCORES       # 512 queries per core
SK = SL + 2 * WIN      # 768 keys/values incl. halo
NQB = SL // 128        # 4 query blocks
NKB = SK // 128        # 6 key blocks
NDB = D // 128         # 8 contraction blocks
NEB = E // 128         # 8 embed blocks
VROW = HD + 1          # 65: v columns per head incl. ones column

# kb-major score tiles: valid q-blocks for key-block kb are
# [max(0, kb-2), min(NQB-1, kb)] (window = +-1 block around diagonal).
KB_Q0 = [max(0, kb - 2) for kb in range(NKB)]
KB_QN = [min(NQB - 1, kb) - max(0, kb - 2) + 1 for kb in range(NKB)]

_CACHE = {}
LAST_RESULT = None  # BassKernelResults of the most recent run (for test.py)


def _build_nc():
    nc = bacc.Bacc("TRN2", target_bir_lowering=False, debug=False)

    qt_d = nc.dram_tensor("qt", [D, SL], BF16, kind="ExternalInput").ap()
    kt_d = nc.dram_tensor("kt", [D, SK], BF16, kind="ExternalInput").ap()
    vt_d = nc.dram_tensor("vt", [D, SK], BF16, kind="ExternalInput").ap()
    wq_d = nc.dram_tensor("wq", [D, E], BF16, kind="ExternalInput").ap()
    wk_d = nc.dram_tensor("wk", [D, E], BF16, kind="ExternalInput").ap()
    wv_d = nc.dram_tensor("wv", [D, E], BF16, kind="ExternalInput").ap()
    wo_d = nc.dram_tensor("wo", [E, D], BF16, kind="ExternalInput").ap()
    # bf16 multiplicative mask pairs (one strided op covers blocks {0,2}
    # of a [128,384] score tile): [0:256]=[mk0|m0] for the kb0+kb1 pair,
    # [256:512]=[m2|m0] for kb2/kb3, [512:768]=[m2|mk5] for the kb4+kb5
    # pair. mk0/mk5 are m0/m2 with the global-edge zero blocks baked in.
    msk_d = nc.dram_tensor("msk", [128, 768], BF16, kind="ExternalInput").ap()
    out_d = nc.dram_tensor("out", [SL, D], FP32, kind="ExternalOutput").ap()

    with tile.TileContext(nc) as tc:
        pools = []

        def pool(name, bufs, **kw):
            p = tc.tile_pool(name=name, bufs=bufs, **kw)
            pools.append(p)
            return p.__enter__()

        const = pool("const", 1)
        psum = pool("psum", 2, space="PSUM")       # proj + out-proj (+vch tag)
        pscore = pool("pscore", 3, space="PSUM")   # score tiles [128, 384]
        ppv_pool = pool("ppv", 2, space="PSUM")    # PV tiles [128, 65]
        ep = pool("expp", 3)                       # per-head exp tiles
        aop = pool("ao", 2)                        # per-qb attn-out [q, e]
        aotp = pool("aot", 16)                     # per-(qb,eb) transposed
        op = pool("o", 4)                          # fp32 out staging [128,256]
        rp = pool("recip", 8)

        # ---- persistent SBUF tensors ----
        warm_sb = const.tile([128, 512], BF16, tag="warm")
        wq_sb = const.tile([128, NDB * E], BF16, tag="wq")
        wk_sb = const.tile([128, NDB * E], BF16, tag="wk")
        wv_sb = const.tile([128, NDB * E], BF16, tag="wv")
        wo_sb = const.tile([128, NEB * D], BF16, tag="wo")
        qtin_sb = const.tile([128, NDB * SL], BF16, tag="qtin")
        ktin_sb = const.tile([128, NDB * SK], BF16, tag="ktin")
        vtin_sb = const.tile([128, NDB * SK], BF16, tag="vtin")
        qt_sb = const.tile([128, NEB * SL], BF16, tag="qt")    # [e,s] per e-blk
        kt_sb = const.tile([128, NEB * SK], BF16, tag="kt")
        v_sb = const.tile([128, NKB * H * VROW], BF16, tag="v")  # [s, h*65]
        msk_sb = const.tile([128, 768], BF16, tag="msk")

        sync = nc.sync

        # ---- PE warm-up: ~4us of dummy matmuls so the HAM clock gate is
        # already 8/8 when the first data-dependent matmul issues ----
        nc.gpsimd.memset(warm_sb[:], 0.0)
        wps = psum.tile([128, 512], FP32, tag="ps")
        for _ in range(10):
            nc.tensor.matmul(
                wps[:], lhsT=warm_sb[:, 0:128], rhs=warm_sb[:],
                start=True, stop=True,
            )

        # ones columns of v_sb (col hd=64 of each head group)
        v3 = v_sb[:].rearrange("p (k h c) -> p k h c", k=NKB, h=H)
        nc.gpsimd.memset(v3[:, :, :, HD:VROW], 1.0)

        # ---- input DMAs: batched 3-D APs on the two HWDGE queues, ordered
        # as the projections consume them ----
        def slabs(dr, ncols):
            return dr.rearrange("(b p) c -> p b c", p=128)

        def sb3(sb, ncols):
            return sb[:].rearrange("p (b c) -> p b c", c=ncols)

        # first Q-proj chain needs qtin slab0 + wq[:, eb0] slabs
        sync.dma_start(qtin_sb[:, 0:SL], qt_d[0:128, :])
        nc.scalar.dma_start(wq_sb[:, 0:128], wq_d[0:128, 0:128])
        nc.scalar.dma_start(
            sb3(wq_sb, E)[:, 1:NDB, 0:128], slabs(wq_d, E)[:, 1:NDB, 0:128]
        )
        sync.dma_start(sb3(qtin_sb, SL)[:, 1:NDB], slabs(qt_d, SL)[:, 1:NDB])
        nc.scalar.dma_start(
            sb3(wq_sb, E)[:, :, 128:E], slabs(wq_d, E)[:, :, 128:E]
        )
        sync.dma_start(sb3(ktin_sb, SK)[:], slabs(kt_d, SK)[:])
        nc.scalar.dma_start(sb3(wk_sb, E)[:], slabs(wk_d, E)[:])
        sync.dma_start(msk_sb[:], msk_d[:])
        nc.scalar.dma_start(sb3(vtin_sb, SK)[:], slabs(vt_d, SK)[:])
        sync.dma_start(sb3(wv_sb, E)[:], slabs(wv_d, E)[:])
        nc.scalar.dma_start(sb3(wo_sb, D)[:], slabs(wo_d, D)[:])

        # ---- q projection: [e, s] = Wq[d,e].T @ QT[d,s] ----
        for eb in range(NEB):
            ps = psum.tile([128, 512], FP32, tag="ps")
            for db in range(NDB):
                nc.tensor.matmul(
                    ps[:],
                    lhsT=wq_sb[:, db * E + eb * 128: db * E + (eb + 1) * 128],
                    rhs=qtin_sb[:, db * SL: db * SL + SL],
                    start=(db == 0),
                    stop=(db == NDB - 1),
                )
            nc.vector.tensor_copy(qt_sb[:, eb * SL:(eb + 1) * SL], ps[:])

        # ---- k projection: [e, s] = Wk[d,e].T @ KT[d,s] over halo range ----
        for eb in range(NEB):
            for s0, s1 in ((0, 512), (512, SK)):
                ps = psum.tile([128, 512], FP32, tag="ps")
                for db in range(NDB):
                    nc.tensor.matmul(
                        ps[:, : s1 - s0],
                        lhsT=wk_sb[:, db * E + eb * 128: db * E + (eb + 1) * 128],
                        rhs=ktin_sb[:, db * SK + s0: db * SK + s1],
                        start=(db == 0),
                        stop=(db == NDB - 1),
                    )
                nc.vector.tensor_copy(
                    kt_sb[:, eb * SK + s0: eb * SK + s1], ps[:, : s1 - s0]
                )

        # ---- v projection chain: one (kb, eh) psum chain ----
        def v_chain(kb, eh):
            ps = psum.tile([128, 512], FP32, tag="vch", bufs=1)
            for db in range(NDB):
                nc.tensor.matmul(
                    ps[:],
                    lhsT=vtin_sb[:, db * SK + kb * 128: db * SK + (kb + 1) * 128],
                    rhs=wv_sb[:, db * E + eh * 512: db * E + (eh + 1) * 512],
                    start=(db == 0),
                    stop=(db == NDB - 1),
                )
            dst = v3[:, kb, eh * 8:(eh + 1) * 8, 0:HD]
            src = ps[:].rearrange("p (h c) -> p h c", c=HD)
            nc.scalar.copy(dst, src)

        # ---- attention pieces ----
        scale = 1.0 / np.sqrt(HD)
        expp_tiles = {}  # (h, kb) -> (sbuf tile, col base)

        def score_pair(j, kbs, mskoff):
            """Score matmuls for head pair (2j, 2j+1) over key blocks kbs.
            The two heads sit at partitions 0-63 / 64-127 of e-block j, so
            per kb their two K=64 matmuls run in different PE row groups
            (concurrently). One exp + one strided mask multiply per head."""
            pscrs = (
                pscore.tile([128, 384], FP32, tag="scr", name=f"scr_{2*j}_{kbs[0]}"),
                pscore.tile([128, 384], FP32, tag="scr", name=f"scr_{2*j+1}_{kbs[0]}"),
            )
            col = 0
            for kb in kbs:
                w = KB_QN[kb] * 128
                q0 = KB_Q0[kb] * 128
                for t in range(2):
                    hp = t * HD
                    nc.tensor.matmul(
                        pscrs[t][:, col:col + w],
                        lhsT=kt_sb[hp:hp + HD,
                                   j * SK + kb * 128: j * SK + (kb + 1) * 128],
                        rhs=qt_sb[hp:hp + HD, j * SL + q0: j * SL + q0 + w],
                        start=True,
                        stop=True,
                    )
                col += w
            for t in range(2):
                h = 2 * j + t
                expp = ep.tile([128, 384], BF16, tag=f"expp{h}",
                               name=f"expp_{h}_{kbs[0]}")
                nc.scalar.activation(
                    expp[:], pscrs[t][:],
                    mybir.ActivationFunctionType.Exp, scale=scale,
                )
                # masked 128-col blocks are always local blocks {0, 2};
                # SBUF-only op, runs on the otherwise-idle GpSimd engine
                ev = expp[:].rearrange("p (b c) -> p b c", b=3)
                dst = ev[:, 0::2]
                src = msk_sb[:, mskoff:mskoff + 256].rearrange(
                    "p (b c) -> p b c", b=2)
                nc.gpsimd.tensor_mul(dst, dst, src)
                col = 0
                for kb in kbs:
                    expp_tiles[(h, kb)] = (expp, col)
                    col += KB_QN[kb] * 128

        def pv_norm(h, qb, ao):
            """PV + normalize for one (head, q-block) into ao tile."""
            ppv = ppv_pool.tile([128, VROW], FP32, tag="pv")
            for r in range(3):
                kb = qb + r
                tile_, base = expp_tiles[(h, kb)]
                off = base + (qb - KB_Q0[kb]) * 128
                nc.tensor.matmul(
                    ppv[:],
                    lhsT=tile_[:, off:off + 128],
                    rhs=v_sb[:, (kb * H + h) * VROW:(kb * H + h + 1) * VROW],
                    start=(r == 0),
                    stop=(r == 2),
                )
            rd = rp.tile([128, 1], FP32, tag="rd")
            nc.vector.reciprocal(rd[:], ppv[:, HD:VROW])
            nc.vector.tensor_scalar(
                ao[:, h * HD:(h + 1) * HD],
                ppv[:, 0:HD],
                rd[:],
                None,
                op0=mybir.AluOpType.mult,
            )

        # ---- prologue attention: kb0+kb1 pair tiles + V kb0..2 ----
        for j in range(H // 2):
            score_pair(j, (0, 1), 0)
            if j == 1:
                v_chain(0, 0)
            if j == 3:
                v_chain(0, 1)
            if j == 5:
                v_chain(1, 0)
            if j == 7:
                v_chain(1, 1)
        v_chain(2, 0)
        v_chain(2, 1)

        # score groups computed per qb iteration (kb4+kb5 paired at qb2)
        KBN_GROUPS = {0: ((2,), 256), 1: ((3,), 256), 2: ((4, 5), 512)}

        # ---- rolling qb loop with per-eb transposes and inline out-proj ----
        for qb in range(NQB):
            ao = aop.tile([128, E], BF16, tag="ao")
            aot = [
                aotp.tile([128, 128], BF16, tag="aot", name=f"aot_{qb}_{eb}")
                for eb in range(NEB)
            ]
            grp = KBN_GROUPS.get(qb)
            pso = [None, None]

            def xpose(eb):
                sync.dma_start_transpose(aot[eb][:], ao[:, eb * 128:(eb + 1) * 128])

            def opj_mm(eb):
                for dh in range(2):
                    if pso[dh] is None:
                        pso[dh] = psum.tile([128, 512], FP32, tag="ps",
                                            name=f"opj_{qb}_{dh}")
                    nc.tensor.matmul(
                        pso[dh][:],
                        lhsT=aot[eb][:],
                        rhs=wo_sb[:, eb * D + dh * 512: eb * D + (dh + 1) * 512],
                        start=(eb == 0),
                        stop=(eb == NEB - 1),
                    )

            for h in range(H):
                if grp is not None and h % 2 == 0:
                    score_pair(h // 2, grp[0], grp[1])
                if qb < NQB - 1 and h in (4, 10):
                    v_chain(qb + 3, 0 if h == 4 else 1)
                if h >= 1:
                    pv_norm(h - 1, qb, ao)
                if h >= 2 and h % 2 == 0:
                    xpose((h - 2) // 2)      # eb 0..6 at h = 2,4,...,14
                if h >= 5 and h % 2 == 1:
                    opj_mm((h - 5) // 2)     # eb 0..5 at h = 5,7,...,15
            pv_norm(H - 1, qb, ao)
            xpose(NEB - 1)
            opj_mm(NEB - 2)
            opj_mm(NEB - 1)
            for dh in range(2):
                for half in range(2):
                    o_t = op.tile([128, 256], FP32, tag="o")
                    src = pso[dh][:, half * 256:(half + 1) * 256]
                    if half == 0:
                        nc.vector.tensor_copy(o_t[:], src)
                    else:
                        nc.scalar.copy(o_t[:], src)
                    eng = sync if half == 0 else nc.scalar
                    eng.dma_start(
                        out_d[qb * 128:(qb + 1) * 128,
                              dh * 512 + half * 256: dh * 512 + (half + 1) * 256],
                        o_t[:],
                    )

        for p in reversed(pools):
            p.__exit__(None, None, None)

    nc.compile()
    return nc


def _host_masks():
    bf = ml_dtypes.bfloat16
    kt = np.arange(128)[:, None]
    qi = np.arange(128)[None, :]
    tri0 = (qi <= kt).astype(bf)          # r=0 keep
    tri2 = (kt <= qi).astype(bf)          # r=2 keep
    zeros = np.zeros((128, 128), bf)

    masks = []
    for c in range(NCORES):
        m = np.empty((128, 768), bf)
        m[:, 0:128] = zeros if c == 0 else tri0           # mk0
        m[:, 128:256] = tri0                              # m0 (kb1 pair)
        m[:, 256:384] = tri2                              # m2 (kb2/kb3)
        m[:, 384:512] = tri0                              # m0 (kb2/kb3)
        m[:, 512:640] = tri2                              # m2 (kb4 pair)
        m[:, 640:768] = zeros if c == NCORES - 1 else tri2  # mk5
        masks.append(m)
    return masks


def _host_inputs(query, key, value, Wq, Wk, Wv, Wo):
    bf = ml_dtypes.bfloat16
    q2 = np.ascontiguousarray(query.reshape(S, D))
    k2 = np.asarray(key).reshape(S, D)
    v2 = np.asarray(value).reshape(S, D)
    kpad = np.zeros((S + 2 * WIN, D), np.float32)
    kpad[WIN:WIN + S] = k2
    vpad = np.zeros((S + 2 * WIN, D), np.float32)
    vpad[WIN:WIN + S] = v2

    wq = np.ascontiguousarray(Wq.astype(bf))
    wk = np.ascontiguousarray(Wk.astype(bf))
    wv = np.ascontiguousarray(Wv.astype(bf))
    wo = np.ascontiguousarray(Wo.astype(bf))
    masks = _host_masks()

    in_maps = []
    for c in range(NCORES):
        s0 = c * SL
        qt = np.ascontiguousarray(q2[s0:s0 + SL].T.astype(bf))
        ktc = np.ascontiguousarray(kpad[s0:s0 + SK].T.astype(bf))
        vtc = np.ascontiguousarray(vpad[s0:s0 + SK].T.astype(bf))
        in_maps.append({
            "qt": qt, "kt": ktc, "vt": vtc,
            "wq": wq, "wk": wk, "wv": wv, "wo": wo,
            "msk": masks[c],
        })
    return in_maps


def kernel(query, key, value, Wq, Wk, Wv, Wo):
    global LAST_RESULT
    if "nc" not in _CACHE:
        _CACHE["nc"] = _build_nc()
    nc = _CACHE["nc"]
    in_maps = _host_inputs(
        np.asarray(query), np.asarray(key), np.asarray(value),
        np.asarray(Wq), np.asarray(Wk), np.asarray(Wv), np.asarray(Wo),
    )
    trace = os.environ.get("KERNEL_TRACE", "0") == "1"
    try:
        res = run_bass_kernel_spmd(
            nc, in_maps, core_ids=list(range(NCORES)), trace=trace
        )
    except ModuleNotFoundError:
        res = run_bass_kernel_spmd(
            nc, in_maps, core_ids=list(range(NCORES)), trace=False
        )
    LAST_RESULT = res
    out = np.concatenate([res.results[c]["out"] for c in range(NCORES)], axis=0)
    return out.reshape(1, S, D).astype(np.float32)


# revision 9
# speedup vs baseline: 1.2857x; 1.0687x over previous
"""LocalAttention (B=1, S=4096, D=1024, H=16, hd=64, window=128) on 8 trn2 cores.

Sharding: sequence-parallel. Core c owns queries [512c, 512c+512) and receives
a key/value halo slice of 768 rows ([512c-128, 512c+640), zero-padded at the
global edges). All projection weights are replicated (bf16). Everything on
device runs in bf16 with fp32 PSUM accumulation.

v4 per-core dataflow:
  PE warm-up (~10 dummy matmuls on a zeroed tile) so the HAM clock gate is
  8/8 when the first real projection fires.  Fine-grained per-slab input
  DMAs ordered exactly as the Q projection consumes them (first matmul
  fires after ~0.4MB instead of ~6MB).
  qT = (Wq^T x^T) in [e, s] layout; kT likewise over the 768-col halo range;
  v = (Vin Wv) in natural [s, e] layout + a ones-column per head (softmax
  denominator rides along col 64 of each head's 65-wide v group).
  Scores issued per HEAD-PAIR: the two heads of an e-block live at
  partitions 0-63 / 64-127, so their K=64 matmuls target different PE
  row-groups and run concurrently.  Exp (scale=1/8, bf16, no max-subtract)
  on ACT; triangle masks on GpSimd.
  PV per (head, q-block): 3 accumulating matmuls lhsT=expp slice,
  rhs=[v_h | 1] -> [128, 65]; DVE reciprocal + tensor_scalar normalize into
  ao ([q, e] bf16).  ao -> aot via [128,256] quarter XBAR transposes (the
  XBAR has ~1.1us fixed cost, so quarters beat finer grains) issued as soon
  as each quarter's 4 heads are normalized; the output projection chain
  matmul for e-block eb issues ~2 heads after its quarter's transpose, so
  the last-qb tail is one transpose + three chain matmuls + split copies.
  Out staging copies are split [128,256] across Vector and Scalar engines,
  DMA'd per 256-col chunk.
"""

import os

import numpy as np
import ml_dtypes

import concourse.bass as bass
import concourse.bacc as bacc
import concourse.mybir as mybir
import concourse.tile as tile
from concourse.bass_utils import run_bass_kernel_spmd

BF16 = mybir.dt.bfloat16
FP32 = mybir.dt.float32

NCORES = 8
S = 4096
D = 1024
H = 16
HD = 64
E = H * HD  # 1024
WIN = 128
SL = S // NCORES       # 512 queries per core
SK = SL + 2 * WIN      # 768 keys/values incl. halo
NQB = SL // 128        # 4 query blocks
NKB = SK // 128        # 6 key blocks
NDB = D // 128         # 8 contraction blocks
NEB = E // 128         # 8 embed blocks
VROW = HD + 1          # 65: v columns per head incl. ones column

# kb-major score tiles: valid q-blocks for key-block kb are
# [max(0, kb-2), min(NQB-1, kb)] (window = +-1 block around diagonal).
KB_Q0 = [max(0, kb - 2) for kb in range(NKB)]
KB_QN = [min(NQB - 1, kb) - max(0, kb - 2) + 1 for kb in range(NKB)]

_CACHE = {}
LAST_RESULT = None  # BassKernelResults of the most recent run (for test.py)


def _build_nc():
    nc = bacc.Bacc("TRN2", target_bir_lowering=False, debug=False)

    qt_d = nc.dram_tensor("qt", [D, SL], BF16, kind="ExternalInput").ap()
    kt_d = nc.dram_tensor("kt", [D, SK], BF16, kind="ExternalInput").ap()
    vt_d = nc.dram_tensor("vt", [D, SK], BF16, kind="ExternalInput").ap()
    wq_d = nc.dram_tensor("wq", [D, E], BF16, kind="ExternalInput").ap()
    wk_d = nc.dram_tensor("wk", [D, E], BF16, kind="ExternalInput").ap()
    wv_d = nc.dram_tensor("wv", [D, E], BF16, kind="ExternalInput").ap()
    wo_d = nc.dram_tensor("wo", [E, D], BF16, kind="ExternalInput").ap()
    # bf16 multiplicative mask pairs (one strided op covers blocks {0,2}
    # of a [128,384] score tile): [0:256]=[mk0|m0] for the kb0+kb1 pair,
    # [256:512]=[m2|m0] for kb2/kb3, [512:768]=[m2|mk5] for the kb4+kb5
    # pair. mk0/mk5 are m0/m2 with the global-edge zero blocks baked in.
    msk_d = nc.dram_tensor("msk", [128, 768], BF16, kind="ExternalInput").ap()
    out_d = nc.dram_tensor("out", [SL, D], FP32, kind="ExternalOutput").ap()

    with tile.TileContext(nc) as tc:
        pools = []

        def pool(name, bufs, **kw):
            p = tc.tile_pool(name=name, bufs=bufs, **kw)
            pools.append(p)
            return p.__enter__()

        const = pool("const", 1)
        psum = pool("psum", 2, space="PSUM")       # proj + out-proj (+vch tag)
        pscore = pool("pscore", 3, space="PSUM")   # score tiles [128, 384]
        ppv_pool = pool("ppv", 2, space="PSUM")    # PV tiles [128, 65]
        ep = pool("expp", 3)                       # per-head exp tiles
        aop = pool("ao", 2)                        # per-qb attn-out [q, e]
        aotp = pool("aot", 16)                     # per-(qb,eb) transposed
        op = pool("o", 4)                          # fp32 out staging [128,256]
        rp = pool("recip", 8)

        # ---- persistent SBUF tensors ----
        warm_sb = const.tile([128, 512], BF16, tag="warm")
        wq_sb = const.tile([128, NDB * E], BF16, tag="wq")
        wk_sb = const.tile([128, NDB * E], BF16, tag="wk")
        wv_sb = const.tile([128, NDB * E], BF16, tag="wv")
        wo_sb = const.tile([128, NEB * D], BF16, tag="wo")
        qtin_sb = const.tile([128, NDB * SL], BF16, tag="qtin")
        ktin_sb = const.tile([128, NDB * SK], BF16, tag="ktin")
        vtin_sb = const.tile([128, NDB * SK], BF16, tag="vtin")
        qt_sb = const.tile([128, NEB * SL], BF16, tag="qt")    # [e,s] per e-blk
        kt_sb = const.tile([128, NEB * SK], BF16, tag="kt")
        v_sb = const.tile([128, NKB * H * VROW], BF16, tag="v")  # [s, h*65]
        msk_sb = const.tile([128, 768], BF16, tag="msk")

        sync = nc.sync

        # ---- PE warm-up: ~4us of dummy matmuls so the HAM clock gate is
        # already 8/8 when the first data-dependent matmul issues ----
        nc.gpsimd.memset(warm_sb[:], 0.0)
        wps = psum.tile([128, 512], FP32, tag="ps")
        for _ in range(10):
            nc.tensor.matmul(
                wps[:], lhsT=warm_sb[:, 0:128], rhs=warm_sb[:],
                start=True, stop=True,
            )

        # ones columns of v_sb (col hd=64 of each head group)
        v3 = v_sb[:].rearrange("p (k h c) -> p k h c", k=NKB, h=H)
        nc.gpsimd.memset(v3[:, :, :, HD:VROW], 1.0)

        # ---- input DMAs: per-slab, ordered as consumed; alternate the two
        # HWDGE engines (SP / Activation) so descriptor issue is parallel ----
        def load_slab(sb, dr, ncols, b):
            eng = sync if b % 2 == 0 else nc.scalar
            eng.dma_start(
                sb[:, b * ncols:(b + 1) * ncols],
                dr[b * 128:(b + 1) * 128],
            )

        # Q-proj data: qtin slab + just the eb0 column block of each wq slab
        # first (all that the first PE chain needs), then the wq remainders.
        for db in range(NDB):
            load_slab(qtin_sb, qt_d, SL, db)
            eng = sync if db % 2 == 0 else nc.scalar
            eng.dma_start(
                wq_sb[:, db * E: db * E + 128],
                wq_d[db * 128:(db + 1) * 128, 0:128],
            )
        for db in range(NDB):
            eng = sync if db % 2 == 0 else nc.scalar
            eng.dma_start(
                wq_sb[:, db * E + 128:(db + 1) * E],
                wq_d[db * 128:(db + 1) * 128, 128:E],
            )
        for db in range(NDB):
            load_slab(ktin_sb, kt_d, SK, db)
            load_slab(wk_sb, wk_d, E, db)
        sync.dma_start(msk_sb[:], msk_d[:])
        for db in range(NDB):
            load_slab(vtin_sb, vt_d, SK, db)
            load_slab(wv_sb, wv_d, E, db)
        for eb in range(NEB):
            load_slab(wo_sb, wo_d, D, eb)

        # ---- q projection: [e, s] = Wq[d,e].T @ QT[d,s] ----
        for eb in range(NEB):
            ps = psum.tile([128, 512], FP32, tag="ps")
            for db in range(NDB):
                nc.tensor.matmul(
                    ps[:],
                    lhsT=wq_sb[:, db * E + eb * 128: db * E + (eb + 1) * 128],
                    rhs=qtin_sb[:, db * SL: db * SL + SL],
                    start=(db == 0),
                    stop=(db == NDB - 1),
                )
            nc.vector.tensor_copy(qt_sb[:, eb * SL:(eb + 1) * SL], ps[:])

        # ---- k projection: [e, s] = Wk[d,e].T @ KT[d,s] over halo range ----
        for eb in range(NEB):
            for s0, s1 in ((0, 512), (512, SK)):
                ps = psum.tile([128, 512], FP32, tag="ps")
                for db in range(NDB):
                    nc.tensor.matmul(
                        ps[:, : s1 - s0],
                        lhsT=wk_sb[:, db * E + eb * 128: db * E + (eb + 1) * 128],
                        rhs=ktin_sb[:, db * SK + s0: db * SK + s1],
                        start=(db == 0),
                        stop=(db == NDB - 1),
                    )
                nc.vector.tensor_copy(
                    kt_sb[:, eb * SK + s0: eb * SK + s1], ps[:, : s1 - s0]
                )

        # ---- v projection chain: one (kb, eh) psum chain ----
        def v_chain(kb, eh):
            ps = psum.tile([128, 512], FP32, tag="vch", bufs=1)
            for db in range(NDB):
                nc.tensor.matmul(
                    ps[:],
                    lhsT=vtin_sb[:, db * SK + kb * 128: db * SK + (kb + 1) * 128],
                    rhs=wv_sb[:, db * E + eh * 512: db * E + (eh + 1) * 512],
                    start=(db == 0),
                    stop=(db == NDB - 1),
                )
            dst = v3[:, kb, eh * 8:(eh + 1) * 8, 0:HD]
            src = ps[:].rearrange("p (h c) -> p h c", c=HD)
            nc.scalar.copy(dst, src)

        # ---- attention pieces ----
        scale = 1.0 / np.sqrt(HD)
        expp_tiles = {}  # (h, kb) -> (sbuf tile, col base)

        def score_pair(j, kbs, mskoff):
            """Score matmuls for head pair (2j, 2j+1) over key blocks kbs.
            The two heads sit at partitions 0-63 / 64-127 of e-block j, so
            per kb their two K=64 matmuls run in different PE row groups
            (concurrently). One exp + one strided mask multiply per head."""
            pscrs = (
                pscore.tile([128, 384], FP32, tag="scr", name=f"scr_{2*j}_{kbs[0]}"),
                pscore.tile([128, 384], FP32, tag="scr", name=f"scr_{2*j+1}_{kbs[0]}"),
            )
            col = 0
            for kb in kbs:
                w = KB_QN[kb] * 128
                q0 = KB_Q0[kb] * 128
                for t in range(2):
                    hp = t * HD
                    nc.tensor.matmul(
                        pscrs[t][:, col:col + w],
                        lhsT=kt_sb[hp:hp + HD,
                                   j * SK + kb * 128: j * SK + (kb + 1) * 128],
                        rhs=qt_sb[hp:hp + HD, j * SL + q0: j * SL + q0 + w],
                        start=True,
                        stop=True,
                    )
                col += w
            for t in range(2):
                h = 2 * j + t
                expp = ep.tile([128, 384], BF16, tag=f"expp{h}",
                               name=f"expp_{h}_{kbs[0]}")
                nc.scalar.activation(
                    expp[:], pscrs[t][:],
                    mybir.ActivationFunctionType.Exp, scale=scale,
                )
                # masked 128-col blocks are always local blocks {0, 2};
                # SBUF-only op, runs on the otherwise-idle GpSimd engine
                ev = expp[:].rearrange("p (b c) -> p b c", b=3)
                dst = ev[:, 0::2]
                src = msk_sb[:, mskoff:mskoff + 256].rearrange(
                    "p (b c) -> p b c", b=2)
                nc.gpsimd.tensor_mul(dst, dst, src)
                col = 0
                for kb in kbs:
                    expp_tiles[(h, kb)] = (expp, col)
                    col += KB_QN[kb] * 128

        def pv_norm(h, qb, ao):
            """PV + normalize for one (head, q-block) into ao tile."""
            ppv = ppv_pool.tile([128, VROW], FP32, tag="pv")
            for r in range(3):
                kb = qb + r
                tile_, base = expp_tiles[(h, kb)]
                off = base + (qb - KB_Q0[kb]) * 128
                nc.tensor.matmul(
                    ppv[:],
                    lhsT=tile_[:, off:off + 128],
                    rhs=v_sb[:, (kb * H + h) * VROW:(kb * H + h + 1) * VROW],
                    start=(r == 0),
                    stop=(r == 2),
                )
            rd = rp.tile([128, 1], FP32, tag="rd")
            nc.vector.reciprocal(rd[:], ppv[:, HD:VROW])
            nc.vector.tensor_scalar(
                ao[:, h * HD:(h + 1) * HD],
                ppv[:, 0:HD],
                rd[:],
                None,
                op0=mybir.AluOpType.mult,
            )

        # ---- prologue attention: kb0+kb1 pair tiles + V kb0..2 ----
        for j in range(H // 2):
            score_pair(j, (0, 1), 0)
            if j == 1:
                v_chain(0, 0)
            if j == 3:
                v_chain(0, 1)
            if j == 5:
                v_chain(1, 0)
            if j == 7:
                v_chain(1, 1)
        v_chain(2, 0)
        v_chain(2, 1)

        # score groups computed per qb iteration (kb4+kb5 paired at qb2)
        KBN_GROUPS = {0: ((2,), 256), 1: ((3,), 256), 2: ((4, 5), 512)}

        # ---- rolling qb loop: quarter transposes + inline per-head out-proj.
        # The XBAR transpose has ~1.1us FIXED cost, so [128,256] quarters
        # (4/qb) beat per-eb [128,128] (8/qb).  The out-proj chain matmul for
        # e-block eb issues ~2 heads after its quarter's transpose so the PE
        # queue never waits on the XBAR. ----
        for qb in range(NQB):
            ao = aop.tile([128, E], BF16, tag="ao")
            aot_q = [
                aotp.tile([128, 256], BF16, tag="aot", name=f"aot_{qb}_{q4}")
                for q4 in range(4)
            ]
            grp = KBN_GROUPS.get(qb)
            pso = [None, None]

            def xpose(q4):
                sync.dma_start_transpose(
                    aot_q[q4][:].rearrange("p (b q) -> p b q", q=128),
                    ao[:, q4 * 256:(q4 + 1) * 256],
                )

            def opj_mm(eb):
                for dh in range(2):
                    if pso[dh] is None:
                        pso[dh] = psum.tile([128, 512], FP32, tag="ps",
                                            name=f"opj_{qb}_{dh}")
                    c0 = (eb % 2) * 128
                    nc.tensor.matmul(
                        pso[dh][:],
                        lhsT=aot_q[eb // 2][:, c0:c0 + 128],
                        rhs=wo_sb[:, eb * D + dh * 512: eb * D + (dh + 1) * 512],
                        start=(eb == 0),
                        stop=(eb == NEB - 1),
                    )

            for h in range(H):
                if grp is not None and h % 2 == 0:
                    score_pair(h // 2, grp[0], grp[1])
                if qb < NQB - 1 and h in (4, 10):
                    v_chain(qb + 3, 0 if h == 4 else 1)
                if h >= 1:
                    pv_norm(h - 1, qb, ao)
                if h in (4, 8, 12):
                    xpose(h // 4 - 1)        # quarters 0..2 at h = 4, 8, 12
                if h >= 7 and h % 2 == 1:
                    opj_mm((h - 7) // 2)     # eb 0..4 at h = 7, 9, ..., 15
            pv_norm(H - 1, qb, ao)
            xpose(3)
            opj_mm(NEB - 3)                  # eb5 fills PE during xpose(3)
            opj_mm(NEB - 2)
            opj_mm(NEB - 1)
            for dh in range(2):
                for half in range(2):
                    o_t = op.tile([128, 256], FP32, tag="o")
                    src = pso[dh][:, half * 256:(half + 1) * 256]
                    if half == 0:
                        nc.vector.tensor_copy(o_t[:], src)
                    else:
                        nc.scalar.copy(o_t[:], src)
                    eng = sync if half == 0 else nc.scalar
                    eng.dma_start(
                        out_d[qb * 128:(qb + 1) * 128,
                              dh * 512 + half * 256: dh * 512 + (half + 1) * 256],
                        o_t[:],
                    )

        for p in reversed(pools):
            p.__exit__(None, None, None)

    nc.compile()
    return nc


def _host_masks():
    bf = ml_dtypes.bfloat16
    kt = np.arange(128)[:, None]
    qi = np.arange(128)[None, :]
    tri0 = (qi <= kt).astype(bf)          # r=0 keep
    tri2 = (kt <= qi).astype(bf)          # r=2 keep
    zeros = np.zeros((128, 128), bf)

    masks = []
    for c in range(NCORES):
        m = np.empty((128, 768), bf)
        m[:, 0:128] = zeros if c == 0 else tri0           # mk0
        m[:, 128:256] = tri0                              # m0 (kb1 pair)
        m[:, 256:384] = tri2                              # m2 (kb2/kb3)
        m[:, 384:512] = tri0                              # m0 (kb2/kb3)
        m[:, 512:640] = tri2                              # m2 (kb4 pair)
        m[:, 640:768] = zeros if c == NCORES - 1 else tri2  # mk5
        masks.append(m)
    return masks


def _host_inputs(query, key, value, Wq, Wk, Wv, Wo):
    bf = ml_dtypes.bfloat16
    q2 = np.ascontiguousarray(query.reshape(S, D))
    k2 = np.asarray(key).reshape(S, D)
    v2 = np.asarray(value).reshape(S, D)
    kpad = np.zeros((S + 2 * WIN, D), np.float32)
    kpad[WIN:WIN + S] = k2
    vpad = np.zeros((S + 2 * WIN, D), np.float32)
    vpad[WIN:WIN + S] = v2

    wq = np.ascontiguousarray(Wq.astype(bf))
    wk = np.ascontiguousarray(Wk.astype(bf))
    wv = np.ascontiguousarray(Wv.astype(bf))
    wo = np.ascontiguousarray(Wo.astype(bf))
    masks = _host_masks()

    in_maps = []
    for c in range(NCORES):
        s0 = c * SL
        qt = np.ascontiguousarray(q2[s0:s0 + SL].T.astype(bf))
        ktc = np.ascontiguousarray(kpad[s0:s0 + SK].T.astype(bf))
        vtc = np.ascontiguousarray(vpad[s0:s0 + SK].T.astype(bf))
        in_maps.append({
            "qt": qt, "kt": ktc, "vt": vtc,
            "wq": wq, "wk": wk, "wv": wv, "wo": wo,
            "msk": masks[c],
        })
    return in_maps


def kernel(query, key, value, Wq, Wk, Wv, Wo):
    global LAST_RESULT
    if "nc" not in _CACHE:
        _CACHE["nc"] = _build_nc()
    nc = _CACHE["nc"]
    in_maps = _host_inputs(
        np.asarray(query), np.asarray(key), np.asarray(value),
        np.asarray(Wq), np.asarray(Wk), np.asarray(Wv), np.asarray(Wo),
    )
    trace = os.environ.get("KERNEL_TRACE", "0") == "1"
    try:
        res = run_bass_kernel_spmd(
            nc, in_maps, core_ids=list(range(NCORES)), trace=trace
        )
    except ModuleNotFoundError:
        res = run_bass_kernel_spmd(
            nc, in_maps, core_ids=list(range(NCORES)), trace=False
        )
    LAST_RESULT = res
    out = np.concatenate([res.results[c]["out"] for c in range(NCORES)], axis=0)
    return out.reshape(1, S, D).astype(np.float32)
